# revision 8
# baseline (speedup 1.0000x reference)
"""Bidirectional RoPE self-attention (Q is both query and key) on 8 trn2 cores.

Math (per (b,h) pair, T=1024, N=256):
    QR = rope(Q); S = QR @ QR.T / 16; out = softmax(S) @ V

Device strategy (V3 — fp8 DoubleRow everywhere on the PE):
  - 96 (b,h) pairs sharded 12-per-core (batch/head parallel, no comm).
  - RoPE runs on the HOST in fp64; the device receives QR pre-scaled by
    1/4 (folds the 1/sqrt(256) softmax scale) as fp8e4m3 in the
    [channel-pair, even/odd-half, t] deinterleaved layout, so scores are
    one fp8 DoubleRow matmul per (t-tile, s-chunk): K=256 in one pass.
  - exp with a host-computed per-row bias b_t = ln(128) - |QR8_t|^2/16.
    The host knows the exact fp8 QR values, so the device diagonal score
    matches the host's to ~1e-5 and exp lands on exactly 128.0 in fp8
    for every row: the dominant softmax weight quantizes exactly, and
    the fp32 accum row-sum Z stays consistent with the quantized E8.
    Off-diagonal weights (<= a few % of the mass) carry the ~6% fp8
    rounding; E8 is written as fp8 and feeds the second DoubleRow pass.
  - attn @ V, transposed: E8 tiles [t, s] are reused as [s, t] via score
    symmetry; V is fp8 (host-cast), K=256 per DoubleRow matmul. The fp8
    V quantization error on the DOMINANT (near-identity) term is
    corrected exactly: the host sends RT8 = fp8(128*(V - fp8(V)))
    transposed, and the DVE adds it to the PSUM block (the diagonal
    weight is 128/Z ~= 1 after the bias trick).
  - The 1/Z normalization happens on the HOST during unsharding: the
    device returns the unnormalized (po + RT8) in bf16 plus the fp32
    accum column Z [128, 8] per pair; out = po / Z[t].
  - AV accumulates s-chunk-outer into 4 parallel PSUM banks (one per
    (nch, tch) output block), so chunk c's matmuls only need the exps of
    t-tiles 2c/2c+1: AV(i) overlaps pair i's own exp chain and the
    kernel stays scalar-engine-bound with no pipeline tail.
"""

from contextlib import ExitStack

import numpy as np

import concourse.bacc as bacc
import concourse.tile as tile
from concourse import mybir

B, NH, T, N = 8, 12, 1024, 256
NCORES = 8
PAIRS = B * NH // NCORES  # 12 (b,h) pairs per core
F32 = mybir.dt.float32
BF16 = mybir.dt.bfloat16
FP8 = mybir.dt.float8e4
EXP = mybir.ActivationFunctionType.Exp
DR = mybir.MatmulPerfMode.DoubleRow

NTT = T // 128  # 8 t-tiles (= s-chunks) per pair


def build_nc(pairs=PAIRS):
    nc = bacc.Bacc("TRN2", target_bir_lowering=False, debug=False,
                   enable_asserts=False)

    qt = nc.dram_tensor("qt", [pairs, 128, 2, T], FP8, kind="ExternalInput")
    v = nc.dram_tensor("v", [pairs, 128, NTT, N], FP8, kind="ExternalInput")
    rt = nc.dram_tensor("rt", [pairs, 128, 2, T], FP8, kind="ExternalInput")
    bd = nc.dram_tensor("bd", [pairs, 128, NTT], F32, kind="ExternalInput")
    outt = nc.dram_tensor("outt", [pairs, 128, 2, T], BF16, kind="ExternalOutput")
    zd = nc.dram_tensor("zd", [pairs, 128, NTT], F32, kind="ExternalOutput")

    with tile.TileContext(nc) as tc, ExitStack() as ctx:
        qpool = ctx.enter_context(tc.tile_pool(name="q", bufs=3))
        vpool = ctx.enter_context(tc.tile_pool(name="v", bufs=3))
        rpool = ctx.enter_context(tc.tile_pool(name="r", bufs=3))
        bpool = ctx.enter_context(tc.tile_pool(name="b", bufs=3))
        epool = ctx.enter_context(tc.tile_pool(name="e", bufs=2))
        opool = ctx.enter_context(tc.tile_pool(name="o", bufs=2))
        zpool = ctx.enter_context(tc.tile_pool(name="z", bufs=2))
        ps_s = ctx.enter_context(tc.tile_pool(name="ps_s", bufs=2, space="PSUM"))
        ps_q = ctx.enter_context(tc.tile_pool(name="ps_q", bufs=1, space="PSUM"))

        for i in range(pairs):
            q8 = qpool.tile([128, 2 * T], FP8, tag="q8")
            nc.sync.dma_start(q8[:].rearrange("p (k t) -> p k t", k=2), qt[i])
            v8 = vpool.tile([128, NTT * N], FP8, tag="v8")
            nc.gpsimd.dma_start(v8[:].rearrange("p (c n) -> p c n", c=NTT), v[i])
            r8 = rpool.tile([128, 2 * T], FP8, tag="r8")
            nc.gpsimd.dma_start(r8[:].rearrange("p (k t) -> p k t", k=2), rt[i])
            bt = bpool.tile([128, NTT], F32, tag="bt")
            nc.gpsimd.dma_start(bt[:], bd[i])

            q3 = q8[:].rearrange("p (j t) -> p j t", j=2)
            v3 = v8[:].rearrange("p (c n) -> p c n", c=NTT)
            r3 = r8[:].rearrange("p (h t) -> p h t", h=2)
            zacc = zpool.tile([128, NTT], F32, tag="zacc")
            e2 = [epool.tile([128, 2 * T], FP8, tag=f"e{c}", name=f"e{c}")
                  for c in range(NTT // 2)]
            po = [ps_q.tile([128, 512], F32, tag=f"po{b}", name=f"po{b}")
                  for b in range(4)]
            o8 = opool.tile([128, 2 * T], BF16, tag="o8")

            for tt in range(NTT):
                ps = ps_s.tile([128, T], F32, tag="ps")
                for sc in range(T // 512):
                    nc.tensor.matmul(
                        ps[:, sc * 512:(sc + 1) * 512],
                        q3[:, :, tt * 128:(tt + 1) * 128],
                        q3[:, :, sc * 512:(sc + 1) * 512],
                        start=True, stop=True, perf_mode=DR,
                    )
                c, j = tt // 2, tt % 2
                nc.scalar.activation(e2[c][:, j * T:(j + 1) * T], ps[:], EXP,
                                     bias=bt[:, tt:tt + 1],
                                     accum_out=zacc[:, tt:tt + 1])
                # AV s-chunk c only needs e2[c]: interleave it right after
                # the exp that completes e2[c] so AV overlaps the exp chain.
                if j == 1:
                    e3 = e2[c][:].rearrange("p (j t) -> p j t", j=2)
                    for nch in range(2):
                        for tch in range(2):
                            nc.tensor.matmul(
                                po[nch * 2 + tch][:],
                                v3[:, 2 * c:2 * c + 2,
                                   nch * 128:nch * 128 + 128],
                                e3[:, :, tch * 512:(tch + 1) * 512],
                                start=(c == 0), stop=(c == NTT // 2 - 1),
                                perf_mode=DR,
                            )
            nc.gpsimd.dma_start(zd[i], zacc[:])
            for nch in range(2):
                for tch in range(2):
                    off = nch * T + tch * 512
                    nc.vector.tensor_add(o8[:, off:off + 512],
                                         po[nch * 2 + tch][:],
                                         r3[:, nch, tch * 512:(tch + 1) * 512])
            nc.sync.dma_start(outt[i], o8[:].rearrange("p (k t) -> p k t", k=2))

    nc.compile()
    return nc


def host_prep(Q, V, freqs):
    """Returns per-core in_maps for the 8 cores."""
    import ml_dtypes
    fp8 = ml_dtypes.float8_e4m3

    Q = np.asarray(Q, dtype=np.float64)
    V = np.ascontiguousarray(np.asarray(V), dtype=np.float32)
    freqs = np.asarray(freqs, dtype=np.float64).reshape(-1)

    G = B * NH
    Qg = Q.reshape(G, T, N)
    Vg = V.reshape(G, T, N)

    # host rope (fp64) + 1/4 scale, quantize to fp8
    half = freqs[0::2]  # [128] cycles-per-step
    t_col = np.arange(T, dtype=np.float64).reshape(T, 1)
    ang = np.mod(t_col * half.reshape(1, 128), 1.0) * (2.0 * np.pi)
    C, S = np.cos(ang), np.sin(ang)  # [T, 128]
    q0, q1 = Qg[:, :, 0::2], Qg[:, :, 1::2]
    QR8 = np.empty((G, T, N), np.float32)
    QR8[:, :, 0::2] = q0 * C - q1 * S
    QR8[:, :, 1::2] = q1 * C + q0 * S
    QR8 = (QR8 * np.float32(0.25)).astype(fp8)
    QR8f = QR8.astype(np.float32)

    # exp bias: ln(128) - |QR8_t|^2 (the exact device diagonal), [g,128,8]
    diag = np.einsum("gtn,gtn->gt", QR8f, QR8f, optimize=True)
    bias = (np.float32(np.log(128.0)) - diag).astype(np.float32)
    biasg = np.ascontiguousarray(bias.reshape(G, NTT, 128).transpose(0, 2, 1))

    # deinterleaved QR [g, ch-pair, even/odd, t]
    QT = np.empty((G, 128, 2, T), fp8)
    QT[:, :, 0] = QR8[:, :, 0::2].transpose(0, 2, 1)
    QT[:, :, 1] = QR8[:, :, 1::2].transpose(0, 2, 1)

    # V fp8 [g, s%128, s//128, n] and RT8 = fp8(128*(V-V8)) as [g, n%128, nch, t]
    V8 = Vg.astype(fp8)
    Vd = np.ascontiguousarray(
        V8.reshape(G, NTT, 128, N).transpose(0, 2, 1, 3))
    R8 = ((Vg - V8.astype(np.float32)) * np.float32(128.0)).astype(fp8)
    Rd = np.ascontiguousarray(R8.reshape(G, T, 2, 128).transpose(0, 3, 2, 1))

    in_maps = []
    for c in range(NCORES):
        sl = slice(c * PAIRS, (c + 1) * PAIRS)
        in_maps.append({"qt": QT[sl], "v": Vd[sl], "rt": Rd[sl],
                        "bd": biasg[sl]})
    return in_maps


_CACHED_NC = None


def kernel(Q, V, freqs):
    global _CACHED_NC
    from concourse.bass_utils import run_bass_kernel_spmd

    in_maps = host_prep(Q, V, freqs)
    if _CACHED_NC is None:
        _CACHED_NC = build_nc()
    res = run_bass_kernel_spmd(_CACHED_NC, in_maps, list(range(NCORES)))
    # outt [pairs, 128 (n%128), 2 (n//128), T] bf16 unnormalized;
    # zd [pairs, 128 (t%128), 8 (t//128)] fp32 softmax row sums.
    outs = np.concatenate([res.results[c]["outt"] for c in range(NCORES)])
    zs = np.concatenate([res.results[c]["zd"] for c in range(NCORES)])
    full = outs.astype(np.float32).transpose(0, 3, 2, 1)  # [g, T, 2, 128]
    zrow = zs.transpose(0, 2, 1).reshape(B * NH, T, 1, 1)  # Z_t, t-linear
    full = (full / zrow).reshape(B * NH, T, N)  # n = k*128 + p
    return np.ascontiguousarray(full).reshape(B, NH, T, N)


# revision 11
# speedup vs baseline: 1.1313x; 1.1313x over previous
"""Bidirectional RoPE self-attention (Q is both query and key) on 8 trn2 cores.

Math (per (b,h) pair, T=1024, N=256):
    QR = rope(Q); S = QR @ QR.T / 16; out = softmax(S) @ V

Device strategy (V3 — fp8 DoubleRow everywhere on the PE):
  - 96 (b,h) pairs sharded 12-per-core (batch/head parallel, no comm).
  - RoPE runs on the HOST in fp64; the device receives QR pre-scaled by
    1/4 (folds the 1/sqrt(256) softmax scale) as fp8e4m3 in the
    [channel-pair, even/odd-half, t] deinterleaved layout, so scores are
    one fp8 DoubleRow matmul per (t-tile, s-chunk): K=256 in one pass.
  - exp with a host-computed per-row bias b_t = ln(128) - |QR8_t|^2/16.
    The host knows the exact fp8 QR values, so the device diagonal score
    matches the host's to ~1e-5 and exp lands on exactly 128.0 in fp8
    for every row: the dominant softmax weight quantizes exactly, and
    the fp32 accum row-sum Z stays consistent with the quantized E8.
    Off-diagonal weights (<= a few % of the mass) carry the ~6% fp8
    rounding; E8 is written as fp8 and feeds the second DoubleRow pass.
  - attn @ V, transposed: E8 tiles [t, s] are reused as [s, t] via score
    symmetry; V is fp8 (host-cast), K=256 per DoubleRow matmul. The fp8
    V quantization error on the DOMINANT (near-identity) term is
    corrected exactly: the host sends RT8 = fp8(128*(V - fp8(V)))
    transposed, and the DVE adds it to the PSUM block (the diagonal
    weight is 128/Z ~= 1 after the bias trick).
  - The 1/Z normalization happens on the HOST during unsharding: the
    device returns the unnormalized (po + RT8) in bf16 plus the fp32
    accum column Z [128, 8] per pair; out = po / Z[t].
  - AV accumulates s-chunk-outer into 4 parallel PSUM banks (one per
    (nch, tch) output block). AV(i-1)'s chunk groups are interleaved
    between the scores tiles of pair i (their exps finished during pair
    i-1, so they never stall the in-order PE queue), and the last pair's
    AV chases its own exp chain directly, killing the pipeline tail.
  - Z row-sums: scalar accum_out for t-tiles 4..7, DVE fp8 free-axis
    reduces over the E8 tiles for t-tiles 0..3 (offloads half the
    accumulator-read overhead from the critical scalar engine).
"""

from contextlib import ExitStack

import numpy as np

import concourse.bacc as bacc
import concourse.tile as tile
from concourse import mybir

B, NH, T, N = 8, 12, 1024, 256
NCORES = 8
PAIRS = B * NH // NCORES  # 12 (b,h) pairs per core
F32 = mybir.dt.float32
BF16 = mybir.dt.bfloat16
FP8 = mybir.dt.float8e4
EXP = mybir.ActivationFunctionType.Exp
DR = mybir.MatmulPerfMode.DoubleRow

NTT = T // 128  # 8 t-tiles (= s-chunks) per pair


def build_nc(pairs=PAIRS):
    nc = bacc.Bacc("TRN2", target_bir_lowering=False, debug=False,
                   enable_asserts=False)

    qt = nc.dram_tensor("qt", [pairs, 128, 2, T], FP8, kind="ExternalInput")
    v = nc.dram_tensor("v", [pairs, 128, NTT, N], FP8, kind="ExternalInput")
    rt = nc.dram_tensor("rt", [pairs, 128, 2, T], FP8, kind="ExternalInput")
    bd = nc.dram_tensor("bd", [pairs, 128, NTT], F32, kind="ExternalInput")
    outt = nc.dram_tensor("outt", [pairs, 128, 2, T], BF16, kind="ExternalOutput")
    zd = nc.dram_tensor("zd", [pairs, 128, NTT], F32, kind="ExternalOutput")

    with tile.TileContext(nc) as tc, ExitStack() as ctx:
        qpool = ctx.enter_context(tc.tile_pool(name="q", bufs=3))
        vpool = ctx.enter_context(tc.tile_pool(name="v", bufs=3))
        rpool = ctx.enter_context(tc.tile_pool(name="r", bufs=3))
        bpool = ctx.enter_context(tc.tile_pool(name="b", bufs=3))
        epool = ctx.enter_context(tc.tile_pool(name="e", bufs=2))
        opool = ctx.enter_context(tc.tile_pool(name="o", bufs=2))
        zpool = ctx.enter_context(tc.tile_pool(name="z", bufs=2))
        ps_s = ctx.enter_context(tc.tile_pool(name="ps_s", bufs=2, space="PSUM"))
        ps_q = ctx.enter_context(tc.tile_pool(name="ps_q", bufs=1, space="PSUM"))

        state = {}

        def av_chunk(i, c, po):
            v8, r8, _, e2 = state[i]
            v3 = v8[:].rearrange("p (c n) -> p c n", c=NTT)
            e3 = e2[c][:].rearrange("p (j t) -> p j t", j=2)
            for nch in range(2):
                for tch in range(2):
                    nc.tensor.matmul(
                        po[nch * 2 + tch][:],
                        v3[:, 2 * c:2 * c + 2, nch * 128:nch * 128 + 128],
                        e3[:, :, tch * 512:(tch + 1) * 512],
                        start=(c == 0), stop=(c == NTT // 2 - 1),
                        perf_mode=DR,
                    )

        def av_finish(i, po):
            _, r8, zacc, _ = state.pop(i)
            r3 = r8[:].rearrange("p (h t) -> p h t", h=2)
            o8 = opool.tile([128, 2 * T], BF16, tag="o8")
            nc.gpsimd.dma_start(zd[i], zacc[:])
            for nch in range(2):
                for tch in range(2):
                    off = nch * T + tch * 512
                    nc.vector.tensor_add(o8[:, off:off + 512],
                                         po[nch * 2 + tch][:],
                                         r3[:, nch, tch * 512:(tch + 1) * 512])
            nc.sync.dma_start(outt[i], o8[:].rearrange("p (k t) -> p k t", k=2))

        po_prev = None
        for i in range(pairs):
            q8 = qpool.tile([128, 2 * T], FP8, tag="q8")
            nc.sync.dma_start(q8[:].rearrange("p (k t) -> p k t", k=2), qt[i])
            v8 = vpool.tile([128, NTT * N], FP8, tag="v8")
            nc.gpsimd.dma_start(v8[:].rearrange("p (c n) -> p c n", c=NTT), v[i])
            r8 = rpool.tile([128, 2 * T], FP8, tag="r8")
            nc.gpsimd.dma_start(r8[:].rearrange("p (k t) -> p k t", k=2), rt[i])
            bt = bpool.tile([128, NTT], F32, tag="bt")
            nc.gpsimd.dma_start(bt[:], bd[i])

            q3 = q8[:].rearrange("p (j t) -> p j t", j=2)
            zacc = zpool.tile([128, NTT], F32, tag="zacc")
            e2 = [epool.tile([128, 2 * T], FP8, tag=f"e{c}", name=f"e{c}")
                  for c in range(NTT // 2)]
            state[i] = (v8, r8, zacc, e2)
            po = [ps_q.tile([128, 512], F32, tag=f"po{b}", name=f"po{b}")
                  for b in range(4)]

            for tt in range(NTT):
                ps = ps_s.tile([128, T], F32, tag="ps")
                for sc in range(T // 512):
                    nc.tensor.matmul(
                        ps[:, sc * 512:(sc + 1) * 512],
                        q3[:, :, tt * 128:(tt + 1) * 128],
                        q3[:, :, sc * 512:(sc + 1) * 512],
                        start=True, stop=True, perf_mode=DR,
                    )
                c, j = tt // 2, tt % 2
                esl = e2[c][:, j * T:(j + 1) * T]
                if tt < NTT // 2:
                    # Z for early tiles via DVE fp8 reduce (keeps the
                    # accumulator-read off the bottleneck scalar engine)
                    nc.scalar.activation(esl, ps[:], EXP,
                                         bias=bt[:, tt:tt + 1])
                    nc.vector.tensor_reduce(zacc[:, tt:tt + 1], esl,
                                            mybir.AxisListType.X,
                                            mybir.AluOpType.add)
                else:
                    nc.scalar.activation(esl, ps[:], EXP,
                                         bias=bt[:, tt:tt + 1],
                                         accum_out=zacc[:, tt:tt + 1])
                # interleave AV chunk groups of the previous pair (their
                # exps finished last pair, so they never stall the PE);
                # the final pair frontloads pair i-1's AV, then chases its
                # own exp chain (po aliases po_prev via bufs=1, so the two
                # quads must be sequenced, not interleaved)
                if j == 1:
                    if i < pairs - 1:
                        if i > 0:
                            av_chunk(i - 1, c, po_prev)
                            if c == NTT // 2 - 1:
                                av_finish(i - 1, po_prev)
                    elif c < 2:
                        av_chunk(i - 1, 2 * c, po_prev)
                        av_chunk(i - 1, 2 * c + 1, po_prev)
                        if c == 1:
                            av_finish(i - 1, po_prev)
                    elif c == NTT // 2 - 1:
                        for cc in range(NTT // 2):
                            av_chunk(i, cc, po)
                        av_finish(i, po)
            po_prev = po

    nc.compile()
    return nc


def host_prep(Q, V, freqs):
    """Returns per-core in_maps for the 8 cores."""
    import ml_dtypes
    fp8 = ml_dtypes.float8_e4m3

    Q = np.asarray(Q, dtype=np.float64)
    V = np.ascontiguousarray(np.asarray(V), dtype=np.float32)
    freqs = np.asarray(freqs, dtype=np.float64).reshape(-1)

    G = B * NH
    Qg = Q.reshape(G, T, N)
    Vg = V.reshape(G, T, N)

    # host rope (fp64) + 1/4 scale, quantize to fp8
    half = freqs[0::2]  # [128] cycles-per-step
    t_col = np.arange(T, dtype=np.float64).reshape(T, 1)
    ang = np.mod(t_col * half.reshape(1, 128), 1.0) * (2.0 * np.pi)
    C, S = np.cos(ang), np.sin(ang)  # [T, 128]
    q0, q1 = Qg[:, :, 0::2], Qg[:, :, 1::2]
    QR8 = np.empty((G, T, N), np.float32)
    QR8[:, :, 0::2] = q0 * C - q1 * S
    QR8[:, :, 1::2] = q1 * C + q0 * S
    QR8 = (QR8 * np.float32(0.25)).astype(fp8)
    QR8f = QR8.astype(np.float32)

    # exp bias: ln(128) - |QR8_t|^2 (the exact device diagonal), [g,128,8]
    diag = np.einsum("gtn,gtn->gt", QR8f, QR8f, optimize=True)
    bias = (np.float32(np.log(128.0)) - diag).astype(np.float32)
    biasg = np.ascontiguousarray(bias.reshape(G, NTT, 128).transpose(0, 2, 1))

    # deinterleaved QR [g, ch-pair, even/odd, t]
    QT = np.empty((G, 128, 2, T), fp8)
    QT[:, :, 0] = QR8[:, :, 0::2].transpose(0, 2, 1)
    QT[:, :, 1] = QR8[:, :, 1::2].transpose(0, 2, 1)

    # V fp8 [g, s%128, s//128, n] and RT8 = fp8(128*(V-V8)) as [g, n%128, nch, t]
    V8 = Vg.astype(fp8)
    Vd = np.ascontiguousarray(
        V8.reshape(G, NTT, 128, N).transpose(0, 2, 1, 3))
    R8 = ((Vg - V8.astype(np.float32)) * np.float32(128.0)).astype(fp8)
    Rd = np.ascontiguousarray(R8.reshape(G, T, 2, 128).transpose(0, 3, 2, 1))

    in_maps = []
    for c in range(NCORES):
        sl = slice(c * PAIRS, (c + 1) * PAIRS)
        in_maps.append({"qt": QT[sl], "v": Vd[sl], "rt": Rd[sl],
                        "bd": biasg[sl]})
    return in_maps


_CACHED_NC = None


def kernel(Q, V, freqs):
    global _CACHED_NC
    from concourse.bass_utils import run_bass_kernel_spmd

    in_maps = host_prep(Q, V, freqs)
    if _CACHED_NC is None:
        _CACHED_NC = build_nc()
    res = run_bass_kernel_spmd(_CACHED_NC, in_maps, list(range(NCORES)))
    # outt [pairs, 128 (n%128), 2 (n//128), T] bf16 unnormalized;
    # zd [pairs, 128 (t%128), 8 (t//128)] fp32 softmax row sums.
    outs = np.concatenate([res.results[c]["outt"] for c in range(NCORES)])
    zs = np.concatenate([res.results[c]["zd"] for c in range(NCORES)])
    full = outs.astype(np.float32).transpose(0, 3, 2, 1)  # [g, T, 2, 128]
    zrow = zs.transpose(0, 2, 1).reshape(B * NH, T, 1, 1)  # Z_t, t-linear
    full = (full / zrow).reshape(B * NH, T, N)  # n = k*128 + p
    return np.ascontiguousarray(full).reshape(B, NH, T, N)


# revision 16
# speedup vs baseline: 1.1571x; 1.0227x over previous
"""Bidirectional RoPE self-attention (Q is both query and key) on 8 trn2 cores.

Math (per (b,h) pair, T=1024, N=256):
    QR = rope(Q); S = QR @ QR.T / 16; out = softmax(S) @ V

Device strategy (V3 — fp8 DoubleRow everywhere on the PE):
  - 96 (b,h) pairs sharded 12-per-core (batch/head parallel, no comm).
  - RoPE runs on the HOST in fp64; the device receives QR pre-scaled by
    1/4 (folds the 1/sqrt(256) softmax scale) as fp8e4m3 in the
    [channel-pair, even/odd-half, t] deinterleaved layout, so scores are
    one fp8 DoubleRow matmul per (t-tile, s-chunk): K=256 in one pass.
  - exp with a host-computed per-row bias b_t = ln(128) - |QR8_t|^2/16.
    The host knows the exact fp8 QR values, so the device diagonal score
    matches the host's to ~1e-5 and exp lands on exactly 128.0 in fp8
    for every row: the dominant softmax weight quantizes exactly, and
    the fp32 accum row-sum Z stays consistent with the quantized E8.
    Off-diagonal weights (<= a few % of the mass) carry the ~6% fp8
    rounding; E8 is written as fp8 and feeds the second DoubleRow pass.
  - attn @ V, transposed: E8 tiles [t, s] are reused as [s, t] via score
    symmetry; V is fp8 (host-cast), K=256 per DoubleRow matmul. The fp8
    V quantization error on the DOMINANT (near-identity) term is
    corrected exactly: the host sends RT8 = fp8(128*(V - fp8(V)))
    transposed, and the DVE adds it to the PSUM block (the diagonal
    weight is 128/Z ~= 1 after the bias trick).
  - The 1/Z normalization happens on the HOST during unsharding: the
    device returns the unnormalized (po + RT8) in bf16 plus the fp32
    accum column Z [128, 8] per pair; out = po / Z[t].
  - AV accumulates s-chunk-outer into 4 parallel PSUM banks (one per
    (nch, tch) output block). AV(i-1)'s chunk groups are interleaved
    between the scores tiles of pair i (their exps finished during pair
    i-1, so they never stall the in-order PE queue), and the last pair's
    AV chases its own exp chain directly, killing the pipeline tail.
  - Z row-sums: scalar accum_out for t-tiles 4..7, DVE fp8 free-axis
    reduces over the E8 tiles for t-tiles 0..3 (offloads half the
    accumulator-read overhead from the critical scalar engine).
"""

from contextlib import ExitStack

import numpy as np

import concourse.bacc as bacc
import concourse.tile as tile
from concourse import mybir

B, NH, T, N = 8, 12, 1024, 256
NCORES = 8
PAIRS = B * NH // NCORES  # 12 (b,h) pairs per core
F32 = mybir.dt.float32
BF16 = mybir.dt.bfloat16
FP8 = mybir.dt.float8e4
EXP = mybir.ActivationFunctionType.Exp
DR = mybir.MatmulPerfMode.DoubleRow

NTT = T // 128  # 8 t-tiles (= s-chunks) per pair


def build_nc(pairs=PAIRS):
    nc = bacc.Bacc("TRN2", target_bir_lowering=False, debug=False,
                   enable_asserts=False)

    qt = nc.dram_tensor("qt", [pairs, 128, 2, T], FP8, kind="ExternalInput")
    v = nc.dram_tensor("v", [pairs, 128, NTT, N], FP8, kind="ExternalInput")
    rt = nc.dram_tensor("rt", [pairs, 128, 2, T], FP8, kind="ExternalInput")
    bd = nc.dram_tensor("bd", [pairs, 128, NTT], F32, kind="ExternalInput")
    outt = nc.dram_tensor("outt", [pairs, 128, 2, T], BF16, kind="ExternalOutput")
    zd = nc.dram_tensor("zd", [pairs, 128, NTT], F32, kind="ExternalOutput")

    with tile.TileContext(nc) as tc, ExitStack() as ctx:
        qpool = ctx.enter_context(tc.tile_pool(name="q", bufs=3))
        vpool = ctx.enter_context(tc.tile_pool(name="v", bufs=3))
        rpool = ctx.enter_context(tc.tile_pool(name="r", bufs=3))
        bpool = ctx.enter_context(tc.tile_pool(name="b", bufs=3))
        epool = ctx.enter_context(tc.tile_pool(name="e", bufs=2))
        opool = ctx.enter_context(tc.tile_pool(name="o", bufs=2))
        zpool = ctx.enter_context(tc.tile_pool(name="z", bufs=2))
        ps_s = ctx.enter_context(tc.tile_pool(name="ps_s", bufs=2, space="PSUM"))
        ps_q = ctx.enter_context(tc.tile_pool(name="ps_q", bufs=1, space="PSUM"))

        state = {}

        def av_chunk(i, c, po):
            v8, r8, _, e2 = state[i]
            v3 = v8[:].rearrange("p (c n) -> p c n", c=NTT)
            e3 = e2[c][:].rearrange("p (j t) -> p j t", j=2)
            for nch in range(2):
                for tch in range(2):
                    nc.tensor.matmul(
                        po[nch * 2 + tch][:],
                        v3[:, 2 * c:2 * c + 2, nch * 128:nch * 128 + 128],
                        e3[:, :, tch * 512:(tch + 1) * 512],
                        start=(c == 0), stop=(c == NTT // 2 - 1),
                        perf_mode=DR,
                    )

        def av_finish(i, po):
            _, r8, zacc, _ = state.pop(i)
            r3 = r8[:].rearrange("p (h t) -> p h t", h=2)
            o8 = opool.tile([128, 2 * T], BF16, tag="o8")
            nc.gpsimd.dma_start(zd[i], zacc[:])
            for nch in range(2):
                for tch in range(2):
                    off = nch * T + tch * 512
                    nc.vector.tensor_add(o8[:, off:off + 512],
                                         po[nch * 2 + tch][:],
                                         r3[:, nch, tch * 512:(tch + 1) * 512])
                # per-nch half DMA so the drain starts after two adds
                nc.sync.dma_start(outt[i, :, nch, :],
                                  o8[:, nch * T:(nch + 1) * T])

        # all 12 pairs' exp biases in one upfront 48KB load so the first
        # exp never waits behind the big v8/r8 transfers
        btall = bpool.tile([128, pairs * NTT], F32, tag="btall")
        nc.scalar.dma_start(
            btall[:].rearrange("p (i c) -> p i c", i=pairs),
            bd[:, :, :].rearrange("i p c -> p i c"))

        po_prev = None
        for i in range(pairs):
            q8 = qpool.tile([128, 2 * T], FP8, tag="q8")
            nc.sync.dma_start(q8[:].rearrange("p (k t) -> p k t", k=2), qt[i])
            v8 = vpool.tile([128, NTT * N], FP8, tag="v8")
            nc.gpsimd.dma_start(v8[:].rearrange("p (c n) -> p c n", c=NTT), v[i])
            r8 = rpool.tile([128, 2 * T], FP8, tag="r8")
            nc.gpsimd.dma_start(r8[:].rearrange("p (k t) -> p k t", k=2), rt[i])
            q3 = q8[:].rearrange("p (j t) -> p j t", j=2)
            zacc = zpool.tile([128, NTT], F32, tag="zacc")
            e2 = [epool.tile([128, 2 * T], FP8, tag=f"e{c}", name=f"e{c}")
                  for c in range(NTT // 2)]
            state[i] = (v8, r8, zacc, e2)
            po = [ps_q.tile([128, 512], F32, tag=f"po{b}", name=f"po{b}")
                  for b in range(4)]

            for tt in range(NTT):
                ps = ps_s.tile([128, T], F32, tag="ps")
                for sc in range(T // 512):
                    nc.tensor.matmul(
                        ps[:, sc * 512:(sc + 1) * 512],
                        q3[:, :, tt * 128:(tt + 1) * 128],
                        q3[:, :, sc * 512:(sc + 1) * 512],
                        start=True, stop=True, perf_mode=DR,
                    )
                c, j = tt // 2, tt % 2
                esl = e2[c][:, j * T:(j + 1) * T]
                bcol = btall[:, i * NTT + tt:i * NTT + tt + 1]
                if tt < NTT // 2:
                    # Z for early tiles via DVE fp8 reduce (keeps the
                    # accumulator-read off the bottleneck scalar engine)
                    nc.scalar.activation(esl, ps[:], EXP, bias=bcol)
                    nc.vector.tensor_reduce(zacc[:, tt:tt + 1], esl,
                                            mybir.AxisListType.X,
                                            mybir.AluOpType.add)
                else:
                    nc.scalar.activation(esl, ps[:], EXP, bias=bcol,
                                         accum_out=zacc[:, tt:tt + 1])
                # interleave AV chunk groups of the previous pair (their
                # exps finished last pair, so they never stall the PE);
                # the final pair frontloads pair i-1's AV, then chases its
                # own exp chain (po aliases po_prev via bufs=1, so the two
                # quads must be sequenced, not interleaved)
                if j == 1:
                    if i < pairs - 1:
                        if i > 0:
                            av_chunk(i - 1, c, po_prev)
                            if c == NTT // 2 - 1:
                                av_finish(i - 1, po_prev)
                    elif c < 2:
                        # last pair: frontload pair i-1's AV into the first
                        # half so this pair's own AV (which reuses the same
                        # PSUM quad) can start as early as possible
                        av_chunk(i - 1, 2 * c, po_prev)
                        av_chunk(i - 1, 2 * c + 1, po_prev)
                        if c == 1:
                            av_finish(i - 1, po_prev)
                    else:
                        av_chunk(i, c - 2, po)
                        if c == NTT // 2 - 1:
                            av_chunk(i, 2, po)
                            av_chunk(i, 3, po)
                            av_finish(i, po)
            po_prev = po

    nc.compile()
    return nc


def host_prep(Q, V, freqs):
    """Returns per-core in_maps for the 8 cores."""
    import ml_dtypes
    fp8 = ml_dtypes.float8_e4m3

    Q = np.asarray(Q, dtype=np.float64)
    V = np.ascontiguousarray(np.asarray(V), dtype=np.float32)
    freqs = np.asarray(freqs, dtype=np.float64).reshape(-1)

    G = B * NH
    Qg = Q.reshape(G, T, N)
    Vg = V.reshape(G, T, N)

    # host rope (fp64) + 1/4 scale, quantize to fp8
    half = freqs[0::2]  # [128] cycles-per-step
    t_col = np.arange(T, dtype=np.float64).reshape(T, 1)
    ang = np.mod(t_col * half.reshape(1, 128), 1.0) * (2.0 * np.pi)
    C, S = np.cos(ang), np.sin(ang)  # [T, 128]
    q0, q1 = Qg[:, :, 0::2], Qg[:, :, 1::2]
    QR8 = np.empty((G, T, N), np.float32)
    QR8[:, :, 0::2] = q0 * C - q1 * S
    QR8[:, :, 1::2] = q1 * C + q0 * S
    QR8 = (QR8 * np.float32(0.25)).astype(fp8)
    QR8f = QR8.astype(np.float32)

    # exp bias: ln(128) - |QR8_t|^2 (the exact device diagonal), [g,128,8]
    diag = np.einsum("gtn,gtn->gt", QR8f, QR8f, optimize=True)
    bias = (np.float32(np.log(128.0)) - diag).astype(np.float32)
    biasg = np.ascontiguousarray(bias.reshape(G, NTT, 128).transpose(0, 2, 1))

    # deinterleaved QR [g, ch-pair, even/odd, t]
    QT = np.empty((G, 128, 2, T), fp8)
    QT[:, :, 0] = QR8[:, :, 0::2].transpose(0, 2, 1)
    QT[:, :, 1] = QR8[:, :, 1::2].transpose(0, 2, 1)

    # V fp8 [g, s%128, s//128, n] and RT8 = fp8(128*(V-V8)) as [g, n%128, nch, t]
    V8 = Vg.astype(fp8)
    Vd = np.ascontiguousarray(
        V8.reshape(G, NTT, 128, N).transpose(0, 2, 1, 3))
    R8 = ((Vg - V8.astype(np.float32)) * np.float32(128.0)).astype(fp8)
    Rd = np.ascontiguousarray(R8.reshape(G, T, 2, 128).transpose(0, 3, 2, 1))

    in_maps = []
    for c in range(NCORES):
        sl = slice(c * PAIRS, (c + 1) * PAIRS)
        in_maps.append({"qt": QT[sl], "v": Vd[sl], "rt": Rd[sl],
                        "bd": biasg[sl]})
    return in_maps


_CACHED_NC = None


def kernel(Q, V, freqs):
    global _CACHED_NC
    from concourse.bass_utils import run_bass_kernel_spmd

    in_maps = host_prep(Q, V, freqs)
    if _CACHED_NC is None:
        _CACHED_NC = build_nc()
    res = run_bass_kernel_spmd(_CACHED_NC, in_maps, list(range(NCORES)))
    # outt [pairs, 128 (n%128), 2 (n//128), T] bf16 unnormalized;
    # zd [pairs, 128 (t%128), 8 (t//128)] fp32 softmax row sums.
    outs = np.concatenate([res.results[c]["outt"] for c in range(NCORES)])
    zs = np.concatenate([res.results[c]["zd"] for c in range(NCORES)])
    full = outs.astype(np.float32).transpose(0, 3, 2, 1)  # [g, T, 2, 128]
    zrow = zs.transpose(0, 2, 1).reshape(B * NH, T, 1, 1)  # Z_t, t-linear
    full = (full / zrow).reshape(B * NH, T, N)  # n = k*128 + p
    return np.ascontiguousarray(full).reshape(B, NH, T, N)


# revision 22
# speedup vs baseline: 1.1687x; 1.0101x over previous
"""Bidirectional RoPE self-attention (Q is both query and key) on 8 trn2 cores.

Math (per (b,h) pair, T=1024, N=256):
    QR = rope(Q); S = QR @ QR.T / 16; out = softmax(S) @ V

Device strategy (V3 — fp8 DoubleRow everywhere on the PE):
  - 96 (b,h) pairs sharded 12-per-core (batch/head parallel, no comm).
  - RoPE runs on the HOST in fp64; the device receives QR pre-scaled by
    1/4 (folds the 1/sqrt(256) softmax scale) as fp8e4m3 in the
    [channel-pair, even/odd-half, t] deinterleaved layout, so scores are
    one fp8 DoubleRow matmul per (t-tile, s-chunk): K=256 in one pass.
  - exp with a host-computed per-row bias b_t = ln(128) - |QR8_t|^2/16.
    The host knows the exact fp8 QR values, so the device diagonal score
    matches the host's to ~1e-5 and exp lands on exactly 128.0 in fp8
    for every row: the dominant softmax weight quantizes exactly, and
    the fp32 accum row-sum Z stays consistent with the quantized E8.
    Off-diagonal weights (<= a few % of the mass) carry the ~6% fp8
    rounding; E8 is written as fp8 and feeds the second DoubleRow pass.
  - attn @ V, transposed: E8 tiles [t, s] are reused as [s, t] via score
    symmetry; V is fp8 (host-cast), K=256 per DoubleRow matmul. The fp8
    V quantization error on the DOMINANT (near-identity) term is
    corrected exactly: the host sends RT8 = fp8(128*(V - fp8(V)))
    transposed, and the DVE adds it to the PSUM block (the diagonal
    weight is 128/Z ~= 1 after the bias trick).
  - The 1/Z normalization happens on the HOST during unsharding: the
    device returns the unnormalized (po + RT8) in bf16 plus the fp32
    accum column Z [128, 8] per pair; out = po / Z[t].
  - AV accumulates s-chunk-outer into 4 parallel PSUM banks (one per
    (nch, tch) output block). AV(i-1)'s chunk groups are interleaved
    between the scores tiles of pair i (their exps finished during pair
    i-1, so they never stall the in-order PE queue), and the last pair's
    AV chases its own exp chain directly, killing the pipeline tail.
  - Z row-sums: scalar accum_out for t-tiles 4..7, DVE fp8 free-axis
    reduces over the E8 tiles for t-tiles 0..3 (offloads half the
    accumulator-read overhead from the critical scalar engine).
"""

from contextlib import ExitStack

import numpy as np

import concourse.bacc as bacc
import concourse.tile as tile
from concourse import mybir

B, NH, T, N = 8, 12, 1024, 256
NCORES = 8
PAIRS = B * NH // NCORES  # 12 (b,h) pairs per core
F32 = mybir.dt.float32
BF16 = mybir.dt.bfloat16
FP8 = mybir.dt.float8e4
EXP = mybir.ActivationFunctionType.Exp
DR = mybir.MatmulPerfMode.DoubleRow
DRSW = mybir.MatmulPerfMode.DoubleRowSwInterleave

NTT = T // 128  # 8 t-tiles (= s-chunks) per pair


def build_nc(pairs=PAIRS):
    nc = bacc.Bacc("TRN2", target_bir_lowering=False, debug=False,
                   enable_asserts=False)

    qt = nc.dram_tensor("qt", [pairs, 128, 2, T], FP8, kind="ExternalInput")
    qsw = nc.dram_tensor("qsw", [pairs, 128, 2 * T], FP8, kind="ExternalInput")
    vsw = nc.dram_tensor("vsw", [pairs, 128, 2 * T], FP8, kind="ExternalInput")
    rt = nc.dram_tensor("rt", [pairs, 128, 2, T], FP8, kind="ExternalInput")
    bd = nc.dram_tensor("bd", [pairs, 128, NTT], F32, kind="ExternalInput")
    outt = nc.dram_tensor("outt", [pairs, 128, 2, T], BF16, kind="ExternalOutput")
    zd = nc.dram_tensor("zd", [pairs, 128, NTT], F32, kind="ExternalOutput")

    with tile.TileContext(nc) as tc, ExitStack() as ctx:
        qpool = ctx.enter_context(tc.tile_pool(name="q", bufs=3))
        vpool = ctx.enter_context(tc.tile_pool(name="v", bufs=3))
        rpool = ctx.enter_context(tc.tile_pool(name="r", bufs=3))
        bpool = ctx.enter_context(tc.tile_pool(name="b", bufs=3))
        epool = ctx.enter_context(tc.tile_pool(name="e", bufs=2))
        opool = ctx.enter_context(tc.tile_pool(name="o", bufs=2))
        zpool = ctx.enter_context(tc.tile_pool(name="z", bufs=2))
        ps_s = ctx.enter_context(tc.tile_pool(name="ps_s", bufs=2, space="PSUM"))
        ps_q = ctx.enter_context(tc.tile_pool(name="ps_q", bufs=1, space="PSUM"))

        state = {}

        def av_chunk(i, c, po):
            v8, r8, _, e2 = state[i]
            # [p, (c nch) block, m-reversed, k-half] software-interleaved
            v3 = v8[:].rearrange("p (b m i) -> p b m i", b=NTT, i=2)
            e3 = e2[c][:].rearrange("p (j t) -> p j t", j=2)
            for nch in range(2):
                for tch in range(2):
                    nc.tensor.matmul(
                        po[nch * 2 + tch][:],
                        v3[:, 2 * c + nch],
                        e3[:, :, tch * 512:(tch + 1) * 512],
                        start=(c == 0), stop=(c == NTT // 2 - 1),
                        perf_mode=DRSW,
                    )

        def av_finish(i, po):
            _, r8, zacc, _ = state.pop(i)
            r3 = r8[:].rearrange("p (h t) -> p h t", h=2)
            o8 = opool.tile([128, 2 * T], BF16, tag="o8")
            nc.gpsimd.dma_start(zd[i], zacc[:])
            for nch in range(2):
                for tch in range(2):
                    off = nch * T + tch * 512
                    nc.vector.tensor_add(o8[:, off:off + 512],
                                         po[nch * 2 + tch][:],
                                         r3[:, nch, tch * 512:(tch + 1) * 512])
                # per-nch half DMA so the drain starts after two adds
                nc.sync.dma_start(outt[i, :, nch, :],
                                  o8[:, nch * T:(nch + 1) * T])

        # all 12 pairs' exp biases in one upfront 48KB load so the first
        # exp never waits behind the big v8/r8 transfers
        btall = bpool.tile([128, pairs * NTT], F32, tag="btall")
        nc.scalar.dma_start(
            btall[:].rearrange("p (i c) -> p i c", i=pairs),
            bd[:, :, :].rearrange("i p c -> p i c"))

        po_prev = None
        for i in range(pairs):
            q8 = qpool.tile([128, 2 * T], FP8, tag="q8")
            nc.sync.dma_start(q8[:].rearrange("p (k t) -> p k t", k=2), qt[i])
            qs8 = qpool.tile([128, 2 * T], FP8, tag="qs8")
            nc.sync.dma_start(qs8[:], qsw[i])
            v8 = vpool.tile([128, 2 * T], FP8, tag="v8")
            nc.gpsimd.dma_start(v8[:], vsw[i])
            r8 = rpool.tile([128, 2 * T], FP8, tag="r8")
            nc.gpsimd.dma_start(r8[:].rearrange("p (k t) -> p k t", k=2), rt[i])
            q3 = q8[:].rearrange("p (j t) -> p j t", j=2)
            qs3 = qs8[:].rearrange("p (b m i) -> p b m i", b=NTT, i=2)
            zacc = zpool.tile([128, NTT], F32, tag="zacc")
            e2 = [epool.tile([128, 2 * T], FP8, tag=f"e{c}", name=f"e{c}")
                  for c in range(NTT // 2)]
            state[i] = (v8, r8, zacc, e2)
            po = [ps_q.tile([128, 512], F32, tag=f"po{b}", name=f"po{b}")
                  for b in range(4)]

            for tt in range(NTT):
                ps = ps_s.tile([128, T], F32, tag="ps")
                for sc in range(T // 512):
                    nc.tensor.matmul(
                        ps[:, sc * 512:(sc + 1) * 512],
                        qs3[:, tt],
                        q3[:, :, sc * 512:(sc + 1) * 512],
                        start=True, stop=True, perf_mode=DRSW,
                    )
                c, j = tt // 2, tt % 2
                esl = e2[c][:, j * T:(j + 1) * T]
                bcol = btall[:, i * NTT + tt:i * NTT + tt + 1]
                if tt < NTT // 2:
                    # Z for early tiles via DVE fp8 reduce (keeps the
                    # accumulator-read off the bottleneck scalar engine)
                    nc.scalar.activation(esl, ps[:], EXP, bias=bcol)
                    nc.vector.tensor_reduce(zacc[:, tt:tt + 1], esl,
                                            mybir.AxisListType.X,
                                            mybir.AluOpType.add)
                else:
                    nc.scalar.activation(esl, ps[:], EXP, bias=bcol,
                                         accum_out=zacc[:, tt:tt + 1])
                # interleave AV chunk groups of the previous pair (their
                # exps finished last pair, so they never stall the PE);
                # the final pair frontloads pair i-1's AV, then chases its
                # own exp chain (po aliases po_prev via bufs=1, so the two
                # quads must be sequenced, not interleaved)
                if j == 1:
                    if i < pairs - 1:
                        if i > 0:
                            av_chunk(i - 1, c, po_prev)
                            if c == NTT // 2 - 1:
                                av_finish(i - 1, po_prev)
                    elif c < 2:
                        # last pair: frontload pair i-1's AV into the first
                        # half so this pair's own AV (which reuses the same
                        # PSUM quad) can start as early as possible
                        av_chunk(i - 1, 2 * c, po_prev)
                        av_chunk(i - 1, 2 * c + 1, po_prev)
                        if c == 1:
                            av_finish(i - 1, po_prev)
                    else:
                        av_chunk(i, c - 2, po)
                        if c == NTT // 2 - 1:
                            av_chunk(i, 2, po)
                            av_chunk(i, 3, po)
                            av_finish(i, po)
            po_prev = po

    nc.compile()
    return nc


def host_prep(Q, V, freqs):
    """Returns per-core in_maps for the 8 cores."""
    import ml_dtypes
    fp8 = ml_dtypes.float8_e4m3

    Q = np.asarray(Q, dtype=np.float64)
    V = np.ascontiguousarray(np.asarray(V), dtype=np.float32)
    freqs = np.asarray(freqs, dtype=np.float64).reshape(-1)

    G = B * NH
    Qg = Q.reshape(G, T, N)
    Vg = V.reshape(G, T, N)

    # host rope (fp64) + 1/4 scale, quantize to fp8
    half = freqs[0::2]  # [128] cycles-per-step
    t_col = np.arange(T, dtype=np.float64).reshape(T, 1)
    ang = np.mod(t_col * half.reshape(1, 128), 1.0) * (2.0 * np.pi)
    C, S = np.cos(ang), np.sin(ang)  # [T, 128]
    q0, q1 = Qg[:, :, 0::2], Qg[:, :, 1::2]
    QR8 = np.empty((G, T, N), np.float32)
    QR8[:, :, 0::2] = q0 * C - q1 * S
    QR8[:, :, 1::2] = q1 * C + q0 * S
    QR8 = (QR8 * np.float32(0.25)).astype(fp8)
    QR8f = QR8.astype(np.float32)

    # exp bias: ln(128) - |QR8_t|^2 (the exact device diagonal), [g,128,8]
    diag = np.einsum("gtn,gtn->gt", QR8f, QR8f, optimize=True)
    bias = (np.float32(np.log(128.0)) - diag).astype(np.float32)
    biasg = np.ascontiguousarray(bias.reshape(G, NTT, 128).transpose(0, 2, 1))

    # deinterleaved QR [g, ch-pair, even/odd, t]
    QT = np.empty((G, 128, 2, T), fp8)
    QT[:, :, 0] = QR8[:, :, 0::2].transpose(0, 2, 1)
    QT[:, :, 1] = QR8[:, :, 1::2].transpose(0, 2, 1)

    # SwInterleave stationary layout per 128-col block: columns reversed,
    # the two k-halves interleaved per column: [A127 B127 A126 ... B0]
    QSW = np.ascontiguousarray(
        QT.reshape(G, 128, 2, NTT, 128)[..., ::-1]
        .transpose(0, 1, 3, 4, 2).reshape(G, 128, 2 * T))

    # V fp8 [g, s%128, s//128, n] and RT8 = fp8(128*(V-V8)) as [g, n%128, nch, t]
    V8 = Vg.astype(fp8)
    Vd = V8.reshape(G, NTT, 128, N).transpose(0, 2, 1, 3)
    # AV stationary blocks (c, nch): Vd[g, p, 2c+j, 128nch+m] sw-interleaved
    VSW = np.ascontiguousarray(
        Vd.reshape(G, 128, NTT // 2, 2, 2, 128)[..., ::-1]
        .transpose(0, 1, 2, 4, 5, 3).reshape(G, 128, 2 * T))
    R8 = ((Vg - V8.astype(np.float32)) * np.float32(128.0)).astype(fp8)
    Rd = np.ascontiguousarray(R8.reshape(G, T, 2, 128).transpose(0, 3, 2, 1))

    in_maps = []
    for c in range(NCORES):
        sl = slice(c * PAIRS, (c + 1) * PAIRS)
        in_maps.append({"qt": QT[sl], "qsw": QSW[sl], "vsw": VSW[sl],
                        "rt": Rd[sl], "bd": biasg[sl]})
    return in_maps


_CACHED_NC = None


def kernel(Q, V, freqs):
    global _CACHED_NC
    from concourse.bass_utils import run_bass_kernel_spmd

    in_maps = host_prep(Q, V, freqs)
    if _CACHED_NC is None:
        _CACHED_NC = build_nc()
    res = run_bass_kernel_spmd(_CACHED_NC, in_maps, list(range(NCORES)))
    # outt [pairs, 128 (n%128), 2 (n//128), T] bf16 unnormalized;
    # zd [pairs, 128 (t%128), 8 (t//128)] fp32 softmax row sums.
    outs = np.concatenate([res.results[c]["outt"] for c in range(NCORES)])
    zs = np.concatenate([res.results[c]["zd"] for c in range(NCORES)])
    full = outs.astype(np.float32).transpose(0, 3, 2, 1)  # [g, T, 2, 128]
    zrow = zs.transpose(0, 2, 1).reshape(B * NH, T, 1, 1)  # Z_t, t-linear
    full = (full / zrow).reshape(B * NH, T, N)  # n = k*128 + p
    return np.ascontiguousarray(full).reshape(B, NH, T, N)


# revision 25
# speedup vs baseline: 1.2399x; 1.0609x over previous
"""Bidirectional RoPE self-attention (Q is both query and key) on 8 trn2 cores.

Math (per (b,h) pair, T=1024, N=256):
    QR = rope(Q); S = QR @ QR.T / 16; out = softmax(S) @ V

Device strategy (V3 — fp8 DoubleRow everywhere on the PE):
  - 96 (b,h) pairs sharded 12-per-core (batch/head parallel, no comm).
  - RoPE runs on the HOST in fp64; the device receives QR pre-scaled by
    1/4 (folds the 1/sqrt(256) softmax scale) as fp8e4m3 in the
    [channel-pair, even/odd-half, t] deinterleaved layout, so scores are
    one fp8 DoubleRow matmul per (t-tile, s-chunk): K=256 in one pass.
  - exp with a host-computed per-row bias b_t = ln(128) - |QR8_t|^2/16.
    The host knows the exact fp8 QR values, so the device diagonal score
    matches the host's to ~1e-5 and exp lands on exactly 128.0 in fp8
    for every row: the dominant softmax weight quantizes exactly, and
    the fp32 accum row-sum Z stays consistent with the quantized E8.
    Off-diagonal weights (<= a few % of the mass) carry the ~6% fp8
    rounding; E8 is written as fp8 and feeds the second DoubleRow pass.
  - attn @ V, transposed: E8 tiles [t, s] are reused as [s, t] via score
    symmetry; V is fp8 (host-cast), K=256 per DoubleRow matmul. The fp8
    V quantization error on the DOMINANT (near-identity) term is
    corrected exactly: the host sends RT8 = fp8(128*(V - fp8(V)))
    transposed, and the DVE adds it to the PSUM block (the diagonal
    weight is 128/Z ~= 1 after the bias trick).
  - The 1/Z normalization happens on the HOST during unsharding: the
    device returns the unnormalized (po + RT8) in bf16 plus the fp32
    accum column Z [128, 8] per pair; out = po / Z[t].
  - AV accumulates s-chunk-outer into 4 parallel PSUM banks (one per
    (nch, tch) output block). AV(i-1)'s chunk groups are interleaved
    between the scores tiles of pair i (their exps finished during pair
    i-1, so they never stall the in-order PE queue), and the last pair's
    AV chases its own exp chain directly, killing the pipeline tail.
  - Z row-sums: scalar accum_out for t-tiles 4..7, DVE fp8 free-axis
    reduces over the E8 tiles for t-tiles 0..3 (offloads half the
    accumulator-read overhead from the critical scalar engine).
"""

from contextlib import ExitStack

import numpy as np

import concourse.bacc as bacc
import concourse.tile as tile
from concourse import mybir

B, NH, T, N = 8, 12, 1024, 256
NCORES = 8
PAIRS = B * NH // NCORES  # 12 (b,h) pairs per core
F32 = mybir.dt.float32
BF16 = mybir.dt.bfloat16
FP8 = mybir.dt.float8e4
EXP = mybir.ActivationFunctionType.Exp
DR = mybir.MatmulPerfMode.DoubleRow
DRSW = mybir.MatmulPerfMode.DoubleRowSwInterleave

NTT = T // 128  # 8 t-tiles (= s-chunks) per pair


def build_nc(pairs=PAIRS):
    nc = bacc.Bacc("TRN2", target_bir_lowering=False, debug=False,
                   enable_asserts=False)

    qt = nc.dram_tensor("qt", [pairs, 128, 2, T], FP8, kind="ExternalInput")
    qsw = nc.dram_tensor("qsw", [pairs, 128, 2 * T], FP8, kind="ExternalInput")
    vsw = nc.dram_tensor("vsw", [pairs, 128, 2 * T], FP8, kind="ExternalInput")
    rt = nc.dram_tensor("rt", [pairs, 128, 2, T], FP8, kind="ExternalInput")
    bd = nc.dram_tensor("bd", [pairs, 128, NTT], F32, kind="ExternalInput")
    outt = nc.dram_tensor("outt", [pairs, 128, 2, T], BF16, kind="ExternalOutput")
    zd = nc.dram_tensor("zd", [pairs, 128, NTT], F32, kind="ExternalOutput")

    with tile.TileContext(nc) as tc, ExitStack() as ctx:
        qpool = ctx.enter_context(tc.tile_pool(name="q", bufs=3))
        vpool = ctx.enter_context(tc.tile_pool(name="v", bufs=3))
        rpool = ctx.enter_context(tc.tile_pool(name="r", bufs=3))
        bpool = ctx.enter_context(tc.tile_pool(name="b", bufs=3))
        epool = ctx.enter_context(tc.tile_pool(name="e", bufs=2))
        opool = ctx.enter_context(tc.tile_pool(name="o", bufs=2))
        zpool = ctx.enter_context(tc.tile_pool(name="z", bufs=2))
        ps_s = ctx.enter_context(tc.tile_pool(name="ps_s", bufs=2, space="PSUM"))
        ps_q = ctx.enter_context(tc.tile_pool(name="ps_q", bufs=1, space="PSUM"))

        state = {}
        o8s = {}

        def av_mm(i, k, po):
            """AV matmul #k (0..15) for pair i: c-group k//4, block k%4."""
            c, b = k // 4, k % 4
            nch, tch = b // 2, b % 2
            v8, _, _, e2 = state[i]
            # [p, (c nch) block, m-reversed, k-half] software-interleaved
            v3 = v8[:].rearrange("p (b m i) -> p b m i", b=NTT, i=2)
            e3 = e2[c][:].rearrange("p (j t) -> p j t", j=2)
            nc.tensor.matmul(
                po[b][:], v3[:, 2 * c + nch],
                e3[:, :, tch * 512:(tch + 1) * 512],
                start=(c == 0), stop=(c == NTT // 2 - 1),
                perf_mode=DRSW,
            )

        def av_add(i, b, po):
            """Drain AV block b of pair i: RT-correction add + half-DMA."""
            nch, tch = b // 2, b % 2
            _, r8, _, _ = state[i]
            r3 = r8[:].rearrange("p (h t) -> p h t", h=2)
            if b == 0:
                o8s[i] = opool.tile([128, 2 * T], BF16, tag="o8", name="o8")
            o8 = o8s[i]
            off = nch * T + tch * 512
            nc.vector.tensor_add(o8[:, off:off + 512], po[b][:],
                                 r3[:, nch, tch * 512:(tch + 1) * 512])
            if tch == 1:
                nc.sync.dma_start(outt[i, :, nch, :],
                                  o8[:, nch * T:(nch + 1) * T])
            if b == 3:
                o8s.pop(i)

        # all 12 pairs' exp biases in one upfront 48KB load so the first
        # exp never waits behind the big v8/r8 transfers
        btall = bpool.tile([128, pairs * NTT], F32, tag="btall")
        nc.scalar.dma_start(
            btall[:].rearrange("p (i c) -> p i c", i=pairs),
            bd[:, :, :].rearrange("i p c -> p i c"))

        po_prev = None
        for i in range(pairs):
            last = i == pairs - 1
            q8 = qpool.tile([128, 2 * T], FP8, tag="q8")
            nc.sync.dma_start(q8[:].rearrange("p (k t) -> p k t", k=2), qt[i])
            qs8 = qpool.tile([128, 2 * T], FP8, tag="qs8")
            nc.gpsimd.dma_start(qs8[:], qsw[i])
            v8 = vpool.tile([128, 2 * T], FP8, tag="v8")
            nc.gpsimd.dma_start(v8[:], vsw[i])
            r8 = rpool.tile([128, 2 * T], FP8, tag="r8")
            nc.gpsimd.dma_start(r8[:].rearrange("p (k t) -> p k t", k=2), rt[i])
            q3 = q8[:].rearrange("p (j t) -> p j t", j=2)
            qs3 = qs8[:].rearrange("p (b m i) -> p b m i", b=NTT, i=2)
            zacc = zpool.tile([128, NTT], F32, tag="zacc")
            e2 = [epool.tile([128, 2 * T], FP8, tag=f"e{c}", name=f"e{c}")
                  for c in range(NTT // 2)]
            state[i] = (v8, r8, zacc, e2)
            po = [ps_q.tile([128, 512], F32, tag=f"po{b}", name=f"po{b}")
                  for b in range(4)]

            for tt in range(NTT):
                ps = ps_s.tile([128, T], F32, tag="ps")
                for sc in range(T // 512):
                    nc.tensor.matmul(
                        ps[:, sc * 512:(sc + 1) * 512],
                        qs3[:, tt],
                        q3[:, :, sc * 512:(sc + 1) * 512],
                        start=True, stop=True, perf_mode=DRSW,
                    )
                c, j = tt // 2, tt % 2
                esl = e2[c][:, j * T:(j + 1) * T]
                bcol = btall[:, i * NTT + tt:i * NTT + tt + 1]
                if tt < NTT // 2:
                    # Z for early tiles via DVE fp8 reduce (keeps the
                    # accumulator-read off the bottleneck scalar engine);
                    # the reduce is ISSUED at the end of the pair so the
                    # DVE drains the previous pair's AV blocks first.
                    nc.scalar.activation(esl, ps[:], EXP, bias=bcol)
                else:
                    nc.scalar.activation(esl, ps[:], EXP, bias=bcol,
                                         accum_out=zacc[:, tt:tt + 1])
                # 2 AV matmuls of the previous pair per scores slot (their
                # exps finished during pair i-1: no PE dependency stalls,
                # and the 0.56us insertion never starves the exp chain).
                # The last pair packs 4 per slot: pair i-1's AV in slots
                # 0..3, its own (chasing its own exp chain) in slots 4..7.
                if i > 0:
                    ks = (range(4 * tt, 4 * tt + 4) if last and tt < 4
                          else range(2 * tt, 2 * tt + 2) if not last
                          else [])
                    for k in ks:
                        av_mm(i - 1, k, po_prev)
                        if k >= 12:
                            av_add(i - 1, k - 12, po_prev)
                    if last and tt >= 4:
                        for k in range(4 * (tt - 4), 4 * (tt - 4) + 4):
                            av_mm(i, k, po)
                            if k >= 12:
                                av_add(i, k - 12, po)
            # Z reduces for this pair's early tiles + the Z column store
            for rtt in range(NTT // 2):
                cc, jj = rtt // 2, rtt % 2
                nc.vector.tensor_reduce(
                    zacc[:, rtt:rtt + 1], e2[cc][:, jj * T:(jj + 1) * T],
                    mybir.AxisListType.X, mybir.AluOpType.add)
            nc.gpsimd.dma_start(zd[i], zacc[:])
            po_prev = po
            if i > 0:
                state.pop(i - 1)
            if last:
                state.pop(i)

    nc.compile()
    return nc


def host_prep(Q, V, freqs):
    """Returns per-core in_maps for the 8 cores."""
    import ml_dtypes
    fp8 = ml_dtypes.float8_e4m3

    Q = np.asarray(Q, dtype=np.float64)
    V = np.ascontiguousarray(np.asarray(V), dtype=np.float32)
    freqs = np.asarray(freqs, dtype=np.float64).reshape(-1)

    G = B * NH
    Qg = Q.reshape(G, T, N)
    Vg = V.reshape(G, T, N)

    # host rope (fp64) + 1/4 scale, quantize to fp8
    half = freqs[0::2]  # [128] cycles-per-step
    t_col = np.arange(T, dtype=np.float64).reshape(T, 1)
    ang = np.mod(t_col * half.reshape(1, 128), 1.0) * (2.0 * np.pi)
    C, S = np.cos(ang), np.sin(ang)  # [T, 128]
    q0, q1 = Qg[:, :, 0::2], Qg[:, :, 1::2]
    QR8 = np.empty((G, T, N), np.float32)
    QR8[:, :, 0::2] = q0 * C - q1 * S
    QR8[:, :, 1::2] = q1 * C + q0 * S
    QR8 = (QR8 * np.float32(0.25)).astype(fp8)
    QR8f = QR8.astype(np.float32)

    # exp bias: ln(128) - |QR8_t|^2 (the exact device diagonal), [g,128,8]
    diag = np.einsum("gtn,gtn->gt", QR8f, QR8f, optimize=True)
    bias = (np.float32(np.log(128.0)) - diag).astype(np.float32)
    biasg = np.ascontiguousarray(bias.reshape(G, NTT, 128).transpose(0, 2, 1))

    # deinterleaved QR [g, ch-pair, even/odd, t]
    QT = np.empty((G, 128, 2, T), fp8)
    QT[:, :, 0] = QR8[:, :, 0::2].transpose(0, 2, 1)
    QT[:, :, 1] = QR8[:, :, 1::2].transpose(0, 2, 1)

    # SwInterleave stationary layout per 128-col block: columns reversed,
    # the two k-halves interleaved per column: [A127 B127 A126 ... B0]
    QSW = np.ascontiguousarray(
        QT.reshape(G, 128, 2, NTT, 128)[..., ::-1]
        .transpose(0, 1, 3, 4, 2).reshape(G, 128, 2 * T))

    # V fp8 [g, s%128, s//128, n] and RT8 = fp8(128*(V-V8)) as [g, n%128, nch, t]
    V8 = Vg.astype(fp8)
    Vd = V8.reshape(G, NTT, 128, N).transpose(0, 2, 1, 3)
    # AV stationary blocks (c, nch): Vd[g, p, 2c+j, 128nch+m] sw-interleaved
    VSW = np.ascontiguousarray(
        Vd.reshape(G, 128, NTT // 2, 2, 2, 128)[..., ::-1]
        .transpose(0, 1, 2, 4, 5, 3).reshape(G, 128, 2 * T))
    R8 = ((Vg - V8.astype(np.float32)) * np.float32(128.0)).astype(fp8)
    Rd = np.ascontiguousarray(R8.reshape(G, T, 2, 128).transpose(0, 3, 2, 1))

    in_maps = []
    for c in range(NCORES):
        sl = slice(c * PAIRS, (c + 1) * PAIRS)
        in_maps.append({"qt": QT[sl], "qsw": QSW[sl], "vsw": VSW[sl],
                        "rt": Rd[sl], "bd": biasg[sl]})
    return in_maps


_CACHED_NC = None


def kernel(Q, V, freqs):
    global _CACHED_NC
    from concourse.bass_utils import run_bass_kernel_spmd

    in_maps = host_prep(Q, V, freqs)
    if _CACHED_NC is None:
        _CACHED_NC = build_nc()
    res = run_bass_kernel_spmd(_CACHED_NC, in_maps, list(range(NCORES)))
    # outt [pairs, 128 (n%128), 2 (n//128), T] bf16 unnormalized;
    # zd [pairs, 128 (t%128), 8 (t//128)] fp32 softmax row sums.
    outs = np.concatenate([res.results[c]["outt"] for c in range(NCORES)])
    zs = np.concatenate([res.results[c]["zd"] for c in range(NCORES)])
    full = outs.astype(np.float32).transpose(0, 3, 2, 1)  # [g, T, 2, 128]
    zrow = zs.transpose(0, 2, 1).reshape(B * NH, T, 1, 1)  # Z_t, t-linear
    full = (full / zrow).reshape(B * NH, T, N)  # n = k*128 + p
    return np.ascontiguousarray(full).reshape(B, NH, T, N)
